# revision 2
# baseline (speedup 1.0000x reference)
"""Banded local-linear layer (nn_LocalLinearLayer) on 8 trn2 NeuronCores.

out[b, o, c] = sum_p W[o, p] * xpad[b, c, p] + bias[o],  band p in [o, o+25)
xpad = pad of x along L: first/last 12 rows block-copied (not reversed).

Strategy (v4, L-sharded):
  - Shard the 4096 output rows across 8 cores (512 rows each); each core
    loads only its slice of the banded weight (~156 KB) and of xpad
    (536 rows), with the full free dim B*C = 2048.
  - Per core: 4 output tiles of M=128 rows. Tile t contracts xpad rows
    [128t, 128t+152) via TWO accumulating matmuls into one PSUM bank:
    A: K=128 (rows [128t,128t+128) = x tile t), B: K=24 (first 24 rows
    of x tile t+1, or the 24-row tail buffer for t=3). No duplicated x.
  - Free dim processed in N=512 chunks -> 16 chunk pipelines, PSUM
    bufs=8; drain alternates ScalarE activation (with bias) / VectorE
    tensor_scalar_add, output written as fp16 (halves output traffic).
  - fp16 operands, fp32 PSUM accumulation. Input DMAs on the Sync HWDGE
    ring, output DMAs on the Scalar ring. x chunk loads for tiles 0/1
    are interleaved so compute starts ~1 us in.
"""

import sys

for _p in ("/opt/trn_rl_repo",):
    if _p not in sys.path:
        sys.path.insert(0, _p)

import numpy as np

import concourse.bass as bass
import concourse.tile as tile
from concourse import bacc, mybir
from concourse.bass_utils import run_bass_kernel_spmd

L = 4096
WIN = 25
PAD = (WIN - 1) // 2  # 12
PADDED = L + 2 * PAD  # 4120
B = 32
C = 64
NCORES = 8
NF = B * C  # 2048 free dim
RPC = L // NCORES  # 512 output rows per core
M = 128  # output rows per tile
NT = RPC // M  # 4 tiles per core
KB = WIN - 1  # 24 straddle rows
NCH = 4  # free-dim chunks per tile
NCHUNK = NF // NCH  # 512

F32 = mybir.dt.float32
F16 = mybir.dt.float16


def _build_nc():
    nc = bacc.Bacc("TRN2", target_bir_lowering=False, debug=False, num_devices=NCORES)
    xm_d = nc.dram_tensor("xm", [M, NT, NF], F16, kind="ExternalInput").ap()
    xt_d = nc.dram_tensor("xt", [KB, NF], F16, kind="ExternalInput").ap()
    wa_d = nc.dram_tensor("wa", [M, NT, M], F16, kind="ExternalInput").ap()
    wb_d = nc.dram_tensor("wb", [KB, NT, M], F16, kind="ExternalInput").ap()
    bias_d = nc.dram_tensor("bias", [M, NT], F32, kind="ExternalInput").ap()
    out_d = nc.dram_tensor("out", [M, NT, NF], F16, kind="ExternalOutput").ap()

    with tile.TileContext(nc) as tc:
        with (
            tc.tile_pool(name="main", bufs=1) as pool,
            tc.tile_pool(name="ps", bufs=8, space=bass.MemorySpace.PSUM) as pspool,
        ):
            wa_s = pool.tile([M, NT, M], F16)
            wb_s = pool.tile([KB, NT, M], F16)
            bias_s = pool.tile([M, NT], F32)
            xt_s = pool.tile([KB, NF], F16)
            xm_s = pool.tile([M, NT, NF], F16)
            o_s = pool.tile([M, NT, NF], F16)

            nc.sync.dma_start(wa_s[:], wa_d)
            nc.sync.dma_start(wb_s[:], wb_d)
            nc.sync.dma_start(bias_s[:], bias_d)
            # tiles 0/1 interleaved per chunk so tile-0 compute (which
            # needs tile 1's rows for its B-matmul) starts early
            for j in range(NCH):
                for t in (0, 1):
                    sl = slice(j * NCHUNK, (j + 1) * NCHUNK)
                    nc.sync.dma_start(xm_s[:, t, sl], xm_d[:, t, sl])
            for t in (2, 3):
                for j in range(NCH):
                    sl = slice(j * NCHUNK, (j + 1) * NCHUNK)
                    nc.sync.dma_start(xm_s[:, t, sl], xm_d[:, t, sl])
            nc.sync.dma_start(xt_s[:], xt_d)

            for t in range(NT):
                for j in range(NCH):
                    sl = slice(j * NCHUNK, (j + 1) * NCHUNK)
                    ps = pspool.tile([M, NCHUNK], F32)
                    nc.tensor.matmul(
                        ps[:], wa_s[:, t], xm_s[:, t, sl], start=True, stop=False
                    )
                    xb = xm_s[0:KB, t + 1, sl] if t < NT - 1 else xt_s[:, sl]
                    nc.tensor.matmul(
                        ps[:], wb_s[:, t], xb, start=False, stop=True
                    )
                    if (t * NCH + j) % 2 == 0:
                        nc.scalar.activation(
                            o_s[:, t, sl],
                            ps[:],
                            mybir.ActivationFunctionType.Identity,
                            bias=bias_s[:, t : t + 1],
                        )
                    else:
                        nc.vector.tensor_scalar_add(
                            o_s[:, t, sl], ps[:], bias_s[:, t : t + 1]
                        )
                    nc.scalar.dma_start(out_d[:, t, sl], o_s[:, t, sl])

    nc.compile()
    return nc


_NC = None


def _get_nc():
    global _NC
    if _NC is None:
        _NC = _build_nc()
    return _NC


# band mask within a [128 rows, 152 cols] weight block: col j nonzero for
# row m iff j in [m, m+WIN)  (same for every tile/core)
_J = np.arange(M + KB)[None, :]
_M = np.arange(M)[:, None]
_BMASK = ((_J >= _M) & (_J < _M + WIN)).astype(np.float32)


def _make_in_maps(x, W, b):
    x = np.asarray(x, dtype=np.float32)
    W = np.asarray(W, dtype=np.float32)
    b = np.asarray(b, dtype=np.float32)
    xl = np.ascontiguousarray(x.transpose(1, 0, 2)).reshape(L, NF)
    xpad = np.concatenate([xl[:PAD], xl, xl[-PAD:]], 0).astype(np.float16)  # [4120,NF]

    in_maps = []
    for c in range(NCORES):
        r0 = RPC * c
        xm = np.ascontiguousarray(
            xpad[r0 : r0 + RPC].reshape(NT, M, NF).transpose(1, 0, 2)
        )
        xt = np.ascontiguousarray(xpad[r0 + RPC : r0 + RPC + KB])
        wa = np.empty((M, NT, M), np.float16)
        wb = np.empty((KB, NT, M), np.float16)
        for t in range(NT):
            base = r0 + M * t
            blk = W[base : base + M, base : base + M + KB] * _BMASK
            wa[:, t, :] = blk[:, :M].T
            wb[:, t, :] = blk[:, M:].T
        bias = np.ascontiguousarray(b[r0 : r0 + RPC].reshape(NT, M).T)
        in_maps.append({"xm": xm, "xt": xt, "wa": wa, "wb": wb, "bias": bias})
    return in_maps


def _gather(results):
    rows = np.concatenate(
        [
            np.asarray(r["out"]).transpose(1, 0, 2).reshape(RPC, B, C)
            for r in results
        ],
        axis=0,
    )  # [L, B, C]
    return np.ascontiguousarray(rows.transpose(1, 0, 2)).astype(np.float32)


def kernel(x: np.ndarray, W: np.ndarray, b: np.ndarray) -> np.ndarray:
    nc = _get_nc()
    res = run_bass_kernel_spmd(nc, _make_in_maps(x, W, b), list(range(NCORES)))
    return _gather(res.results)


if __name__ == "__main__":
    rng = np.random.default_rng(0)
    x = rng.standard_normal((B, L, C), dtype=np.float32)
    W = rng.standard_normal((L, PADDED), dtype=np.float32) * 0.02
    b = rng.standard_normal((L,), dtype=np.float32) * 0.02
    print(kernel(x, W, b).shape)


# revision 4
# speedup vs baseline: 1.0696x; 1.0696x over previous
"""Banded local-linear layer (nn_LocalLinearLayer) on 8 trn2 NeuronCores.

out[b, o, c] = sum_p W[o, p] * xpad[b, c, p] + bias[o],  band p in [o, o+25)
xpad = pad of x along L: first/last 12 rows block-copied (not reversed).

Strategy (v5, L-sharded, halo tiles):
  - Shard the 4096 output rows across 8 cores (512 rows each); each core
    loads only its slice of the banded weight (~133 KB) and of xpad
    (536 rows incl. halo), with the full free dim B*C = 2048.
  - Per core: 5 output tiles of M=104 rows (last 96). Tile t loads xpad
    rows [104t, 104t+128) (24-row halo overlap) -> ONE K=128 matmul per
    (tile, 1024-col chunk): 10 matmuls total, N=1024 moving.
  - bf16 operands and bf16 output (fp32 PSUM accumulation).
  - Input DMAs split across both HWDGE rings: weights + x tiles 0-2 on
    Sync, bias + x tiles 3-4 on Scalar (which later carries outputs).
  - PSUM->SBUF drain (+bias) alternates ScalarE activation / VectorE
    tensor_scalar_add per chunk.
  - A short burst of throwaway matmuls on zeroed scratch runs during the
    DMA lead-in to lift the PE out of its cold 1.2 GHz HAM p-state.
"""

import sys

for _p in ("/opt/trn_rl_repo",):
    if _p not in sys.path:
        sys.path.insert(0, _p)

import ml_dtypes
import numpy as np

import concourse.bass as bass
import concourse.tile as tile
from concourse import bacc, mybir
from concourse.bass_utils import run_bass_kernel_spmd

L = 4096
WIN = 25
PAD = 12
PADDED = L + 2 * PAD  # 4120
B = 32
C = 64
NCORES = 8
NF = B * C  # 2048 free dim
RPC = L // NCORES  # 512 output rows per core
M = 104  # output rows per tile (128 - 24 halo)
NT = 5  # tiles per core; last tile has 96 rows / K=120
HALO = WIN - 1  # 24
NCH = 2  # free-dim chunks per tile
NCHUNK = NF // NCH  # 1024
NWARM = 4  # throwaway PE warm-up matmuls

F32 = mybir.dt.float32
BF16 = mybir.dt.bfloat16
NPBF16 = ml_dtypes.bfloat16


def _mt(t):
    return M if t < NT - 1 else RPC - M * (NT - 1)  # 96 for the last tile


def _kt(t):
    return min(128, RPC + HALO - M * t)  # 120 for the last tile


def _build_nc():
    nc = bacc.Bacc("TRN2", target_bir_lowering=False, debug=False, num_devices=NCORES)
    xm_d = nc.dram_tensor("xm", [128, NT, NF], BF16, kind="ExternalInput").ap()
    wa_d = nc.dram_tensor("wa", [128, NT, M], BF16, kind="ExternalInput").ap()
    bias_d = nc.dram_tensor("bias", [M, NT], F32, kind="ExternalInput").ap()
    out_d = nc.dram_tensor("out", [M, NT, NF], BF16, kind="ExternalOutput").ap()

    with tile.TileContext(nc) as tc:
        with (
            tc.tile_pool(name="main", bufs=1) as pool,
            tc.tile_pool(name="ps", bufs=3, space=bass.MemorySpace.PSUM) as pspool,
            tc.tile_pool(name="psw", bufs=1, space=bass.MemorySpace.PSUM) as pswarm,
        ):
            wa_s = pool.tile([128, NT, M], BF16)
            bias_s = pool.tile([M, NT], F32)
            xm_s = pool.tile([128, NT, NF], BF16)
            o_s = pool.tile([M, NT, NF], BF16)
            warm_s = pool.tile([128, 512], BF16)

            # PE warm-up: harmless matmuls on zeroed scratch while input
            # DMAs stream in, so the HAM clock gate opens before real work
            nc.gpsimd.memset(warm_s[:], 0)
            for _ in range(NWARM):
                pz = pswarm.tile([128, 512], F32)
                nc.tensor.matmul(
                    pz[:], warm_s[:, :128], warm_s[:], start=True, stop=True
                )

            nc.sync.dma_start(wa_s[:], wa_d)
            for t in range(3):
                for j in range(NCH):
                    sl = slice(j * NCHUNK, (j + 1) * NCHUNK)
                    nc.sync.dma_start(xm_s[:, t, sl], xm_d[:, t, sl])
            nc.scalar.dma_start(bias_s[:], bias_d)
            for t in range(3, NT):
                kt = _kt(t)
                for j in range(NCH):
                    sl = slice(j * NCHUNK, (j + 1) * NCHUNK)
                    nc.scalar.dma_start(xm_s[:kt, t, sl], xm_d[:kt, t, sl])

            for t in range(NT):
                mt, kt = _mt(t), _kt(t)
                for j in range(NCH):
                    sl = slice(j * NCHUNK, (j + 1) * NCHUNK)
                    ps = pspool.tile([M, NCHUNK], F32)
                    # one matmul per 512-col half: a single matmul's PSUM
                    # output may not cross a 2 KB bank boundary
                    for h in range(2):
                        hsl = slice(j * NCHUNK + h * 512, j * NCHUNK + (h + 1) * 512)
                        nc.tensor.matmul(
                            ps[:mt, h * 512 : (h + 1) * 512],
                            wa_s[:kt, t, :mt],
                            xm_s[:kt, t, hsl],
                            start=True,
                            stop=True,
                        )
                    if (t * NCH + j) % 2 == 0:
                        nc.scalar.activation(
                            o_s[:mt, t, sl],
                            ps[:mt],
                            mybir.ActivationFunctionType.Identity,
                            bias=bias_s[:mt, t : t + 1],
                        )
                    else:
                        nc.vector.tensor_scalar_add(
                            o_s[:mt, t, sl], ps[:mt], bias_s[:mt, t : t + 1]
                        )
                    nc.scalar.dma_start(out_d[:mt, t, sl], o_s[:mt, t, sl])

    nc.compile()
    return nc


_NC = None


def _get_nc():
    global _NC
    if _NC is None:
        _NC = _build_nc()
    return _NC


# band mask within a [104 rows, 128 cols] weight block: col k nonzero for
# row m iff k in [m, m+WIN)  (same for every tile/core)
_K = np.arange(128)[None, :]
_MM = np.arange(M)[:, None]
_BMASK = ((_K >= _MM) & (_K < _MM + WIN)).astype(np.float32)


def _make_in_maps(x, W, b):
    x = np.asarray(x, dtype=np.float32)
    W = np.asarray(W, dtype=np.float32)
    b = np.asarray(b, dtype=np.float32)
    xl = np.ascontiguousarray(x.transpose(1, 0, 2)).reshape(L, NF)
    xpad = np.concatenate([xl[:PAD], xl, xl[-PAD:]], 0).astype(NPBF16)  # [4120,NF]

    in_maps = []
    for c in range(NCORES):
        r0 = RPC * c
        xm = np.zeros((128, NT, NF), NPBF16)
        wa = np.zeros((128, NT, M), NPBF16)
        bias = np.zeros((M, NT), np.float32)
        for t in range(NT):
            mt, kt = _mt(t), _kt(t)
            o0 = r0 + M * t
            xm[:kt, t] = xpad[o0 : o0 + kt]
            blk = W[o0 : o0 + mt, o0 : o0 + kt] * _BMASK[:mt, :kt]
            wa[:kt, t, :mt] = blk.T
            bias[:mt, t] = b[o0 : o0 + mt]
        in_maps.append({"xm": xm, "wa": wa, "bias": bias})
    return in_maps


def _gather_core(out_arr):
    """[104, NT, 2048] bf16 -> [512, B, C] f32 rows for one core."""
    rows = np.empty((RPC, B, C), np.float32)
    for t in range(NT):
        mt = _mt(t)
        rows[M * t : M * t + mt] = (
            out_arr[:mt, t].astype(np.float32).reshape(mt, B, C)
        )
    return rows


def _gather(results):
    rows = np.concatenate(
        [_gather_core(np.asarray(r["out"])) for r in results], axis=0
    )  # [L, B, C]
    return np.ascontiguousarray(rows.transpose(1, 0, 2))


def kernel(x: np.ndarray, W: np.ndarray, b: np.ndarray) -> np.ndarray:
    nc = _get_nc()
    res = run_bass_kernel_spmd(nc, _make_in_maps(x, W, b), list(range(NCORES)))
    return _gather(res.results)


if __name__ == "__main__":
    rng = np.random.default_rng(0)
    x = rng.standard_normal((B, L, C), dtype=np.float32)
    W = rng.standard_normal((L, PADDED), dtype=np.float32) * 0.02
    b = rng.standard_normal((L,), dtype=np.float32) * 0.02
    print(kernel(x, W, b).shape)


# revision 5
# speedup vs baseline: 1.1310x; 1.0574x over previous
"""Banded local-linear layer (nn_LocalLinearLayer) on 8 trn2 NeuronCores.

out[b, o, c] = sum_p W[o, p] * xpad[b, c, p] + bias[o],  band p in [o, o+25)
xpad = pad of x along L: first/last 12 rows block-copied (not reversed).

Strategy (v5, L-sharded, halo tiles):
  - Shard the 4096 output rows across 8 cores (512 rows each); each core
    loads only its slice of the banded weight (~133 KB) and of xpad
    (536 rows incl. halo), with the full free dim B*C = 2048.
  - Per core: 5 output tiles of M=104 rows (last 96). Tile t loads xpad
    rows [104t, 104t+128) (24-row halo overlap) -> ONE K=128 matmul per
    (tile, 1024-col chunk): 10 matmuls total, N=1024 moving.
  - bf16 operands and bf16 output (fp32 PSUM accumulation).
  - Input DMAs split across both HWDGE rings: weights + x tiles 0-2 on
    Sync, bias + x tiles 3-4 on Scalar (which later carries outputs).
  - PSUM->SBUF drain (+bias) alternates ScalarE activation / VectorE
    tensor_scalar_add per chunk.
  - A short burst of throwaway matmuls on zeroed scratch runs during the
    DMA lead-in to lift the PE out of its cold 1.2 GHz HAM p-state.
"""

import sys

for _p in ("/opt/trn_rl_repo",):
    if _p not in sys.path:
        sys.path.insert(0, _p)

import ml_dtypes
import numpy as np

import concourse.bass as bass
import concourse.tile as tile
from concourse import bacc, mybir
from concourse.bass_utils import run_bass_kernel_spmd

L = 4096
WIN = 25
PAD = 12
PADDED = L + 2 * PAD  # 4120
B = 32
C = 64
NCORES = 8
NF = B * C  # 2048 free dim
RPC = L // NCORES  # 512 output rows per core
M = 104  # output rows per tile (128 - 24 halo)
NT = 5  # tiles per core; last tile has 96 rows / K=120
HALO = WIN - 1  # 24
NCH = 2  # free-dim chunks per tile
NCHUNK = NF // NCH  # 1024
NWARM = 4  # throwaway PE warm-up matmuls

F32 = mybir.dt.float32
BF16 = mybir.dt.bfloat16
NPBF16 = ml_dtypes.bfloat16


def _mt(t):
    return M if t < NT - 1 else RPC - M * (NT - 1)  # 96 for the last tile


def _kt(t):
    return min(128, RPC + HALO - M * t)  # 120 for the last tile


def _build_nc():
    nc = bacc.Bacc("TRN2", target_bir_lowering=False, debug=False, num_devices=NCORES)
    xm_d = nc.dram_tensor("xm", [128, NT, NF], BF16, kind="ExternalInput").ap()
    wa_d = nc.dram_tensor("wa", [128, NT, M], BF16, kind="ExternalInput").ap()
    bias_d = nc.dram_tensor("bias", [M, NT], F32, kind="ExternalInput").ap()
    out_d = nc.dram_tensor("out", [M, NT, NF], BF16, kind="ExternalOutput").ap()

    with tile.TileContext(nc) as tc:
        with (
            tc.tile_pool(name="main", bufs=1) as pool,
            tc.tile_pool(name="ps", bufs=3, space=bass.MemorySpace.PSUM) as pspool,
            tc.tile_pool(name="psw", bufs=1, space=bass.MemorySpace.PSUM) as pswarm,
        ):
            wa_s = pool.tile([128, NT, M], BF16)
            bias_s = pool.tile([M, NT], F32)
            xm_s = pool.tile([128, NT, NF], BF16)
            o_s = pool.tile([M, NT, NF], BF16)
            warm_s = pool.tile([128, 512], BF16)

            # PE warm-up: harmless matmuls on zeroed scratch while input
            # DMAs stream in, so the HAM clock gate opens before real work
            nc.gpsimd.memset(warm_s[:], 0)
            for _ in range(NWARM):
                pz = pswarm.tile([128, 512], F32)
                nc.tensor.matmul(
                    pz[:], warm_s[:, :128], warm_s[:], start=True, stop=True
                )

            def load_x(t, j):
                kt = _kt(t)
                sl = slice(j * NCHUNK, (j + 1) * NCHUNK)
                nc.sync.dma_start(xm_s[:kt, t, sl], xm_d[:kt, t, sl])

            nc.sync.dma_start(wa_s[:], wa_d)
            nc.scalar.dma_start(bias_s[:], bias_d)
            # prefetch tile 0 and half of tile 1; the rest is interleaved
            # into the compute loop (program order bounds how much input
            # DMA any compute instruction's semaphore wait can alias with)
            load_x(0, 0)
            load_x(0, 1)
            load_x(1, 0)

            for t in range(NT):
                mt, kt = _mt(t), _kt(t)
                for j in range(NCH):
                    # stay ~1.5 tiles of x ahead of the compute
                    nxt = t * NCH + j + 3
                    if nxt < NT * NCH:
                        load_x(nxt // NCH, nxt % NCH)
                    sl = slice(j * NCHUNK, (j + 1) * NCHUNK)
                    ps = pspool.tile([M, NCHUNK], F32)
                    # one matmul per 512-col half: a single matmul's PSUM
                    # output may not cross a 2 KB bank boundary
                    for h in range(2):
                        hsl = slice(j * NCHUNK + h * 512, j * NCHUNK + (h + 1) * 512)
                        nc.tensor.matmul(
                            ps[:mt, h * 512 : (h + 1) * 512],
                            wa_s[:kt, t, :mt],
                            xm_s[:kt, t, hsl],
                            start=True,
                            stop=True,
                        )
                    if (t * NCH + j) % 2 == 0:
                        nc.scalar.activation(
                            o_s[:mt, t, sl],
                            ps[:mt],
                            mybir.ActivationFunctionType.Identity,
                            bias=bias_s[:mt, t : t + 1],
                        )
                    else:
                        nc.vector.tensor_scalar_add(
                            o_s[:mt, t, sl], ps[:mt], bias_s[:mt, t : t + 1]
                        )
                    nc.scalar.dma_start(out_d[:mt, t, sl], o_s[:mt, t, sl])

    nc.compile()
    return nc


_NC = None


def _get_nc():
    global _NC
    if _NC is None:
        _NC = _build_nc()
    return _NC


# band mask within a [104 rows, 128 cols] weight block: col k nonzero for
# row m iff k in [m, m+WIN)  (same for every tile/core)
_K = np.arange(128)[None, :]
_MM = np.arange(M)[:, None]
_BMASK = ((_K >= _MM) & (_K < _MM + WIN)).astype(np.float32)


def _make_in_maps(x, W, b):
    x = np.asarray(x, dtype=np.float32)
    W = np.asarray(W, dtype=np.float32)
    b = np.asarray(b, dtype=np.float32)
    xl = np.ascontiguousarray(x.transpose(1, 0, 2)).reshape(L, NF)
    xpad = np.concatenate([xl[:PAD], xl, xl[-PAD:]], 0).astype(NPBF16)  # [4120,NF]

    in_maps = []
    for c in range(NCORES):
        r0 = RPC * c
        xm = np.zeros((128, NT, NF), NPBF16)
        wa = np.zeros((128, NT, M), NPBF16)
        bias = np.zeros((M, NT), np.float32)
        for t in range(NT):
            mt, kt = _mt(t), _kt(t)
            o0 = r0 + M * t
            xm[:kt, t] = xpad[o0 : o0 + kt]
            blk = W[o0 : o0 + mt, o0 : o0 + kt] * _BMASK[:mt, :kt]
            wa[:kt, t, :mt] = blk.T
            bias[:mt, t] = b[o0 : o0 + mt]
        in_maps.append({"xm": xm, "wa": wa, "bias": bias})
    return in_maps


def _gather_core(out_arr):
    """[104, NT, 2048] bf16 -> [512, B, C] f32 rows for one core."""
    rows = np.empty((RPC, B, C), np.float32)
    for t in range(NT):
        mt = _mt(t)
        rows[M * t : M * t + mt] = (
            out_arr[:mt, t].astype(np.float32).reshape(mt, B, C)
        )
    return rows


def _gather(results):
    rows = np.concatenate(
        [_gather_core(np.asarray(r["out"])) for r in results], axis=0
    )  # [L, B, C]
    return np.ascontiguousarray(rows.transpose(1, 0, 2))


def kernel(x: np.ndarray, W: np.ndarray, b: np.ndarray) -> np.ndarray:
    nc = _get_nc()
    res = run_bass_kernel_spmd(nc, _make_in_maps(x, W, b), list(range(NCORES)))
    return _gather(res.results)


if __name__ == "__main__":
    rng = np.random.default_rng(0)
    x = rng.standard_normal((B, L, C), dtype=np.float32)
    W = rng.standard_normal((L, PADDED), dtype=np.float32) * 0.02
    b = rng.standard_normal((L,), dtype=np.float32) * 0.02
    print(kernel(x, W, b).shape)


# revision 9
# speedup vs baseline: 1.2006x; 1.0616x over previous
"""Banded local-linear layer (nn_LocalLinearLayer) on 8 trn2 NeuronCores.

out[b, o, c] = sum_p W[o, p] * xpad[b, c, p] + bias[o],  band p in [o, o+25)
xpad = pad of x along L: first/last 12 rows block-copied (not reversed).

Strategy (v7, L-sharded, halo tiles, per-chunk tiles):
  - Shard the 4096 output rows across 8 cores (512 rows each); each core
    loads only its slice of the banded weight (~133 KB) and of xpad
    (536 rows incl. halo), with the full free dim B*C = 2048.
  - Per core: 5 output tiles of M=104 rows (last 96). Tile t loads xpad
    rows [104t, 104t+128) (24-row halo duplicated in the DRAM layout) ->
    one K=128 matmul per (tile, 512-col half): 20 matmuls.
  - Every x chunk / output chunk is its OWN SBUF tile object: Tile
    dependency tracking is tile-granular, so shared big tiles serialize
    the pipeline behind unrelated DMA writes.
  - bf16 operands and bf16 output (fp32 PSUM accumulation).
  - Input DMAs on the Sync ring, bias + output DMAs on the Scalar ring.
  - PSUM->SBUF drain (+bias) per 1024-col chunk alternates VectorE
    tensor_scalar_add / ScalarE activation (ACT, the faster one, takes
    the last chunk which sits on the critical tail).
  - A burst of throwaway matmuls on zeroed scratch runs during the DMA
    lead-in to keep the PE pipeline hot from the first real matmul.
"""

import sys

for _p in ("/opt/trn_rl_repo",):
    if _p not in sys.path:
        sys.path.insert(0, _p)

import ml_dtypes
import numpy as np

import concourse.bass as bass
import concourse.tile as tile
from concourse import bacc, mybir
from concourse.bass_utils import run_bass_kernel_spmd

L = 4096
WIN = 25
PAD = 12
PADDED = L + 2 * PAD  # 4120
B = 32
C = 64
NCORES = 8
NF = B * C  # 2048 free dim
RPC = L // NCORES  # 512 output rows per core
M = 104  # output rows per tile (128 - 24 halo)
NT = 5  # tiles per core; last tile has 96 rows / K=120
HALO = WIN - 1  # 24
NCH = 2  # free-dim chunks per tile
NCHUNK = NF // NCH  # 1024
NWARM = 4  # throwaway PE warm-up matmuls

F32 = mybir.dt.float32
BF16 = mybir.dt.bfloat16
NPBF16 = ml_dtypes.bfloat16


def _mt(t):
    return M if t < NT - 1 else RPC - M * (NT - 1)  # 96 for the last tile


def _kt(t):
    return min(128, RPC + HALO - M * t)  # 120 for the last tile


def _build_nc():
    nc = bacc.Bacc("TRN2", target_bir_lowering=False, debug=False, num_devices=NCORES)
    xm_d = nc.dram_tensor("xm", [128, NT, NF], BF16, kind="ExternalInput").ap()
    wa_d = nc.dram_tensor("wa", [128, NT, M], BF16, kind="ExternalInput").ap()
    bias_d = nc.dram_tensor("bias", [M, NT], F32, kind="ExternalInput").ap()
    out_d = nc.dram_tensor("out", [M, NT, NF], BF16, kind="ExternalOutput").ap()

    with tile.TileContext(nc) as tc:
        with (
            tc.tile_pool(name="main", bufs=1) as pool,
            tc.tile_pool(name="ps", bufs=3, space=bass.MemorySpace.PSUM) as pspool,
            tc.tile_pool(name="psw", bufs=1, space=bass.MemorySpace.PSUM) as pswarm,
        ):
            wa_s = pool.tile([128, NT, M], BF16)
            bias_s = pool.tile([M, NT], F32)
            warm_s = pool.tile([128, 512], BF16)
            xs = [
                [
                    pool.tile([_kt(t), NCHUNK], BF16, name=f"x{t}_{j}")
                    for j in range(NCH)
                ]
                for t in range(NT)
            ]
            os_ = [
                [
                    pool.tile([_mt(t), NCHUNK], BF16, name=f"o{t}_{j}")
                    for j in range(NCH)
                ]
                for t in range(NT)
            ]

            # PE warm-up: harmless matmuls on zeroed scratch while input
            # DMAs stream in, so the PE pipeline is hot for real work
            nc.gpsimd.memset(warm_s[:], 0)
            for _ in range(NWARM):
                pz = pswarm.tile([128, 512], F32)
                nc.tensor.matmul(
                    pz[:], warm_s[:, :128], warm_s[:], start=True, stop=True
                )

            nc.sync.dma_start(wa_s[:], wa_d)
            nc.scalar.dma_start(bias_s[:], bias_d)
            for t in range(NT):
                for j in range(NCH):
                    sl = slice(j * NCHUNK, (j + 1) * NCHUNK)
                    nc.sync.dma_start(xs[t][j][:], xm_d[: _kt(t), t, sl])

            for t in range(NT):
                mt, kt = _mt(t), _kt(t)
                for j in range(NCH):
                    sl = slice(j * NCHUNK, (j + 1) * NCHUNK)
                    ps = pspool.tile([M, NCHUNK], F32)
                    # one matmul per 512-col half: a single matmul's PSUM
                    # output may not cross a 2 KB bank boundary
                    for h in range(2):
                        nc.tensor.matmul(
                            ps[:mt, h * 512 : (h + 1) * 512],
                            wa_s[:kt, t, :mt],
                            xs[t][j][:, h * 512 : (h + 1) * 512],
                            start=True,
                            stop=True,
                        )
                    if (t * NCH + j) % 2 == 1:
                        nc.scalar.activation(
                            os_[t][j][:],
                            ps[:mt],
                            mybir.ActivationFunctionType.Identity,
                            bias=bias_s[:mt, t : t + 1],
                        )
                    else:
                        nc.vector.tensor_scalar_add(
                            os_[t][j][:], ps[:mt], bias_s[:mt, t : t + 1]
                        )
                    nc.scalar.dma_start(out_d[:mt, t, sl], os_[t][j][:])

    nc.compile()
    return nc


_NC = None


def _get_nc():
    global _NC
    if _NC is None:
        _NC = _build_nc()
    return _NC


# band mask within a [104 rows, 128 cols] weight block: col k nonzero for
# row m iff k in [m, m+WIN)  (same for every tile/core)
_K = np.arange(128)[None, :]
_MM = np.arange(M)[:, None]
_BMASK = ((_K >= _MM) & (_K < _MM + WIN)).astype(np.float32)


def _make_in_maps(x, W, b):
    x = np.asarray(x, dtype=np.float32)
    W = np.asarray(W, dtype=np.float32)
    b = np.asarray(b, dtype=np.float32)
    xl = np.ascontiguousarray(x.transpose(1, 0, 2)).reshape(L, NF)
    xpad = np.concatenate([xl[:PAD], xl, xl[-PAD:]], 0).astype(NPBF16)  # [4120,NF]

    in_maps = []
    for c in range(NCORES):
        r0 = RPC * c
        xm = np.zeros((128, NT, NF), NPBF16)
        wa = np.zeros((128, NT, M), NPBF16)
        bias = np.zeros((M, NT), np.float32)
        for t in range(NT):
            mt, kt = _mt(t), _kt(t)
            o0 = r0 + M * t
            xm[:kt, t] = xpad[o0 : o0 + kt]
            blk = W[o0 : o0 + mt, o0 : o0 + kt] * _BMASK[:mt, :kt]
            wa[:kt, t, :mt] = blk.T
            bias[:mt, t] = b[o0 : o0 + mt]
        in_maps.append({"xm": xm, "wa": wa, "bias": bias})
    return in_maps


def _gather_core(out_arr):
    """[104, NT, 2048] bf16 -> [512, B, C] f32 rows for one core."""
    rows = np.empty((RPC, B, C), np.float32)
    for t in range(NT):
        mt = _mt(t)
        rows[M * t : M * t + mt] = (
            out_arr[:mt, t].astype(np.float32).reshape(mt, B, C)
        )
    return rows


def _gather(results):
    rows = np.concatenate(
        [_gather_core(np.asarray(r["out"])) for r in results], axis=0
    )  # [L, B, C]
    return np.ascontiguousarray(rows.transpose(1, 0, 2))


def kernel(x: np.ndarray, W: np.ndarray, b: np.ndarray) -> np.ndarray:
    nc = _get_nc()
    res = run_bass_kernel_spmd(nc, _make_in_maps(x, W, b), list(range(NCORES)))
    return _gather(res.results)


if __name__ == "__main__":
    rng = np.random.default_rng(0)
    x = rng.standard_normal((B, L, C), dtype=np.float32)
    W = rng.standard_normal((L, PADDED), dtype=np.float32) * 0.02
    b = rng.standard_normal((L,), dtype=np.float32) * 0.02
    print(kernel(x, W, b).shape)
